# revision 36
# baseline (speedup 1.0000x reference)
"""MicroGPT forward pass on 8 Trainium2 NeuronCores (Bass/Tile).

Sharding: token-sharded — core c = 2*b + h owns batch b, sequence half h
(512 contiguous tokens). Activations are feature-major in SBUF
(x^T: [768 rows -> 6 tiles of 128, 512 token cols]); all matmuls fp32r/f16.
LN gains are folded into the consumer weights host-side; LN betas become
bias columns added at PSUM eviction (Q/K via activation bias, V via an
extra ones-row matmul, fc1 via the gelu activation bias, unembed host-side).
Per-token LN stats (1/std, mean/std) are broadcast across partitions with
ones-row matmuls into PSUM (no DRAM roundtrips); softmax reciprocals
likewise via a 2-row selector matmul per head pair.
Attention: S^T = K^T-slice (stationary) x Q^T (moving); softmax without max
subtraction (scores bounded); denominators via a ones column appended to V.
K/V slots 0-3 are the core's own 4 blocks (block-causal masks on GpSimd);
slots 4-7 are the pair core's blocks, weighted by a per-core 0/1 scalar.
Per layer a PAIR-group AllGather ([[0,1],[2,3],[4,5],[6,7]]) shares K^T/V;
warmup collectives at program start absorb the NRT rendezvous cost.
Final token: masked 8-way AllReduce, then final LN + vocab-sharded unembed
(4000 vocab rows per core); unembed bias applied host-side.
"""
import sys, math

sys.path.insert(0, "/opt/trn_rl_repo")
import numpy as np

import concourse.bass as bass
import concourse.bacc as bacc
import concourse.mybir as mybir
import concourse.tile as tile

D, NH, DH, FF, NL, V = 768, 12, 64, 3072, 4, 32000
B, S = 4, 1024
EPS = 1e-5
NC_ = 8
P = 128
T = 512            # tokens per core
DT = D // P        # 6 d-model tiles
FT = FF // P       # 24 ff tiles
KB = 8             # key slots (0-3 own, 4-7 pair)
VS = V // NC_      # 4000 vocab rows per core
VCH = 8            # vocab chunks of 500
VCW = VS // VCH    # 500
F32 = mybir.dt.float32
F32R = mybir.dt.float32r
F16 = mybir.dt.float16
BF16 = mybir.dt.bfloat16
I32 = mybir.dt.int32
AF = mybir.ActivationFunctionType
OP = mybir.AluOpType
SCALE = 1.0 / math.sqrt(DH)
VW = NH * (DH + 1)           # 780 — V tile width incl. ones cols
CONTRIB_W = DT * T + 4 * VW  # AllGather contribution width
PAIRS = [[0, 1], [2, 3], [4, 5], [6, 7]]


# ---------------------------------------------------------------- bass program
def build_nc(n_layers=NL, pcol=511, dbg=False):
    nc = bacc.Bacc(None, target_bir_lowering=False, debug=False, num_devices=NC_)

    x0T = nc.dram_tensor("x0T", [DT, P, T], F32R, kind="ExternalInput")
    wqT = nc.dram_tensor("wqT", [n_layers, P, DT * D], F16, kind="ExternalInput")
    wkT = nc.dram_tensor("wkT", [n_layers, P, DT * D], F16, kind="ExternalInput")
    wvT = nc.dram_tensor("wvT", [n_layers, P, DT * D], F16, kind="ExternalInput")
    woT = nc.dram_tensor("woT", [n_layers, P, DT * D], F16, kind="ExternalInput")
    fc1T = nc.dram_tensor("fc1T", [n_layers, FT, P, DT * P], F16, kind="ExternalInput")
    fc2T = nc.dram_tensor("fc2T", [n_layers, FT, P, D], F16, kind="ExternalInput")
    qkb = nc.dram_tensor("qkb", [n_layers, P, 2 * DT], F32, kind="ExternalInput")
    vb = nc.dram_tensor("vb", [n_layers, 1, NH * DH], F16, kind="ExternalInput")
    fb = nc.dram_tensor("fb", [n_layers, P, FT], F32, kind="ExternalInput")
    uT = nc.dram_tensor("uT", [DT, P, VS], F16, kind="ExternalInput")
    masks = nc.dram_tensor("masks", [P, P], F16, kind="ExternalInput")
    remw = nc.dram_tensor("remw", [P, 1], F32, kind="ExternalInput")
    sel4 = nc.dram_tensor("sel4", [P, B], F32R, kind="ExternalInput")
    pairsel = nc.dram_tensor("pairsel", [1, 1], I32, kind="ExternalInput")
    bc2d = nc.dram_tensor("bc2d", [2, P], F32R, kind="ExternalInput")

    out = nc.dram_tensor("out", [B, VS], F32, kind="ExternalOutput")

    with tile.TileContext(nc) as tc:
        with (
            tc.tile_pool(name="const", bufs=1) as cpool,
            tc.tile_pool(name="persist", bufs=1) as ppool,
            tc.tile_pool(name="xp", bufs=6) as xpool,
            tc.tile_pool(name="hp", bufs=7) as hpool,
            tc.tile_pool(name="qp", bufs=6) as qpool,
            tc.tile_pool(name="ac", bufs=6) as apool,
            tc.tile_pool(name="wp", bufs=2) as wpool,
            tc.tile_pool(name="fp", bufs=3) as fpool,
            tc.tile_pool(name="ep", bufs=5) as epool,
            tc.tile_pool(name="sp", bufs=5) as spool,
            tc.tile_pool(name="up", bufs=28) as upool,
            tc.tile_pool(name="rp", bufs=3) as rpool,
            tc.tile_pool(name="psm", bufs=6, space="PSUM") as psm,
            tc.tile_pool(name="psr", bufs=2, space="PSUM") as psr,
            tc.tile_pool(name="dram", bufs=2, space="DRAM") as dpool,
        ):
            # pair rank register for dynamic reads of the AllGather output.
            # MUST precede the warmup collectives: the critical section
            # serializes the Sync engine (which issues all DMAs) against all
            # prior work, so putting it after the warmups stalls every input
            # DMA until the collective rendezvous completes (~50us).
            with tc.tile_critical():
                with nc.sync.register("pairreg") as preg:
                    nc.sync.reg_load(preg, pairsel[0:1, 0:1])
                    pv = nc.sync.snap(preg, min_val=0, max_val=1)

            # ---- warmup collectives: absorb NRT rendezvous cost while the
            # input DMAs + L0 LN/KV run. Same shapes of groups as the real ops.
            wg_in = dpool.tile([P, 8], F16, tag="wua", name="wg_in")
            wg_out = dpool.tile([2, P, 8], F16, tag="wub", name="wg_out")
            nc.gpsimd.collective_compute(
                "AllGather", OP.bypass,
                ins=[wg_in[:].opt()], outs=[wg_out[:].opt()],
                replica_groups=PAIRS,
            )
            # (no AllReduce warmup: the one-time NRT barrier is absorbed by
            # the pair-AG warmup above, and an AR trigger would head-block
            # the GpSimd queue — delaying L0's K AllGather by ~50us)

            # ---- constants (memset cannot write f32r; stage via f32 + copy)
            ones_f32 = cpool.tile([P, 1], F32)
            nc.vector.memset(ones_f32[:], 1.0)
            trimask = cpool.tile([P, P], F16)
            nc.sync.dma_start(trimask[:], masks[:])
            ones_col = cpool.tile([P, 1], F32R)
            nc.vector.tensor_copy(ones_col[:], ones_f32[:])
            onesr_f32 = cpool.tile([1, P], F32)
            nc.vector.memset(onesr_f32[:], 1.0)
            ones_row = cpool.tile([1, P], F32R)
            nc.vector.tensor_copy(ones_row[:], onesr_f32[:])
            ones_row_h = cpool.tile([1, P], F16)
            nc.vector.tensor_copy(ones_row_h[:], onesr_f32[:])
            bc2 = cpool.tile([2, P], F32R)
            nc.sync.dma_start(bc2[:], bc2d[:])
            eps1 = cpool.tile([1, 1], F32)
            nc.vector.memset(eps1[:], EPS)
            sel4_sb = cpool.tile([P, B], F32R)
            nc.sync.dma_start(sel4_sb[:], sel4[:])
            remw_sb = cpool.tile([P, 1], F32)
            nc.sync.dma_start(remw_sb[:], remw[:])

            # persistent K^T / V buffers (slots 0-3 own, 4-7 pair)
            KT = [ppool.tile([P, KB * P], F16, tag=f"kt{e}", name=f"KT{e}")
                  for e in range(DT)]
            VT = [ppool.tile([P, VW], F16, tag=f"vt{j}", name=f"VT{j}")
                  for j in range(KB)]
            for j in range(4):
                for h in range(NH):
                    nc.vector.tensor_copy(
                        VT[j][:, h * (DH + 1) + DH : h * (DH + 1) + DH + 1],
                        ones_f32[:])

            # ---- residual stream (updated in place by residual adds)
            xT = []
            for k in range(DT):
                t_ = xpool.tile([P, T], F32R, tag="xT", name=f"xT{k}")
                nc.sync.dma_start(t_[:], x0T[k])
                xT.append(t_)

            def layer_norm(l, lo=0):
                """Stats via ones-column matmuls; rstd/mrs broadcast across
                partitions via ones-row matmuls into PSUM. Returns y tiles
                (normalized WITHOUT gain/bias — those are folded into the
                consumer weights). With lo>0, only token columns [lo:T] are
                processed; returned tiles hold them at local cols [0:T-lo]."""
                W = T - lo
                sum_ps = psm.tile([1, T], F32, tag="acc", space="PSUM", name="sum_ps")
                sq_ps = psm.tile([1, T], F32, tag="acc", space="PSUM", name="sq_ps")
                for k in range(DT):
                    xsq = epool.tile([P, T], F32R, tag="lntmp", name="xsq")
                    nc.vector.tensor_mul(xsq[:, 0:W], xT[k][:, lo:T], xT[k][:, lo:T])
                    nc.tensor.matmul(sum_ps[:, 0:W], ones_col[:], xT[k][:, lo:T],
                                     start=(k == 0), stop=(k == DT - 1))
                    nc.tensor.matmul(sq_ps[:, 0:W], ones_col[:], xsq[:, 0:W],
                                     start=(k == 0), stop=(k == DT - 1))
                sums_sb = spool.tile([1, T], F32, tag="lnstat", name="sums_sb")
                nc.vector.tensor_copy(sums_sb[:, 0:W], sum_ps[:, 0:W])
                m2s = spool.tile([1, T], F32, tag="lnstat", name="m2s")
                nc.vector.scalar_tensor_tensor(out=m2s[:, 0:W], in0=sums_sb[:, 0:W],
                                               scalar=1.0 / (D * D), in1=sums_sb[:, 0:W],
                                               op0=OP.mult, op1=OP.mult)
                var = spool.tile([1, T], F32, tag="lnstat", name="var")
                nc.vector.scalar_tensor_tensor(out=var[:, 0:W], in0=sq_ps[:, 0:W],
                                               scalar=1.0 / D, in1=m2s[:, 0:W],
                                               op0=OP.mult, op1=OP.subtract)
                rstd = spool.tile([1, T], F32, tag="lnr", name="rstd")
                nc.scalar.activation(rstd[:, 0:W], var[:, 0:W],
                                     AF.Abs_reciprocal_sqrt, bias=eps1[:])
                rstd_r = spool.tile([1, T], F32R, tag="lnr", name="rstd_r")
                nc.vector.tensor_copy(rstd_r[:, 0:W], rstd[:, 0:W])
                mrs = spool.tile([1, T], F32R, tag="lnr", name="mrs")
                nc.vector.scalar_tensor_tensor(out=mrs[:, 0:W], in0=sums_sb[:, 0:W],
                                               scalar=1.0 / D, in1=rstd[:, 0:W],
                                               op0=OP.mult, op1=OP.mult)
                rb_ps = psr.tile([P, T], F32, tag="rot", space="PSUM", name="rb_ps")
                nc.tensor.matmul(rb_ps[:, 0:W], ones_row[:], rstd_r[:, 0:W],
                                 start=True, stop=True)
                mb_ps = psr.tile([P, T], F32, tag="rot", space="PSUM", name="mb_ps")
                nc.tensor.matmul(mb_ps[:, 0:W], ones_row[:], mrs[:, 0:W],
                                 start=True, stop=True)
                hT = []
                for k in range(DT):
                    t1 = epool.tile([P, T], F32R, tag="lntmp", name="lnt1")
                    nc.vector.tensor_mul(t1[:, 0:W], xT[k][:, lo:T], rb_ps[:, 0:W])
                    h_ = hpool.tile([P, T], F16, tag="hT", name="hT_t")
                    nc.vector.tensor_sub(h_[:, 0:W], t1[:, 0:W], mb_ps[:, 0:W])
                    hT.append(h_)
                return hT

            for l in range(n_layers):
                with nc.named_scope(f"L{l}"):
                    # Last layer: only the 128-col query block holding pcol
                    # feeds the output — restrict all query-side work to it.
                    # (K/V stay full width: every key is still attended.)
                    q0 = (pcol // P) * P if l == n_layers - 1 else 0
                    QW = T - q0
                    bqk = spool.tile([P, 2 * DT], F32, tag="lngb", name="bqk")
                    nc.sync.dma_start(bqk[:], qkb[l])
                    vb_sb = spool.tile([1, NH * DH], F16, tag="vbr", name="vb_sb")
                    nc.sync.dma_start(vb_sb[:], vb[l])
                    fb_sb = spool.tile([P, FT], F32, tag="fbg", name="fb_sb")
                    nc.sync.dma_start(fb_sb[:], fb[l])

                    hT = layer_norm(l)

                    # ---- K^T first: its pair AllGather launches while V/Q
                    # are still computing (remote K is needed first, by QK)
                    wk_sb = wpool.tile([P, DT * D], F16, tag="w", name="wk_sb")
                    nc.sync.dma_start(wk_sb[:], wkT[l])
                    for m in range(DT):
                        k_ps = psr.tile([P, T], F32, tag="rot", space="PSUM", name="k_ps")
                        for k in range(DT):
                            nc.tensor.matmul(
                                k_ps[:], wk_sb[:, k * D + m * P : k * D + (m + 1) * P],
                                hT[k][:], start=(k == 0), stop=(k == DT - 1))
                        nc.scalar.activation(KT[m][:, 0:T], k_ps[:], AF.Identity,
                                             bias=bqk[:, DT + m : DT + m + 1])
                    kcon = dpool.tile([P, DT * T], F16, tag="kcon", name="kcon")
                    for e in range(DT):
                        nc.sync.dma_start(kcon[:, e * T : (e + 1) * T],
                                          KT[e][:, 0:T])
                    kout = dpool.tile([2, P, DT * T], F16, tag="kout", name="kout")
                    nc.gpsimd.collective_compute(
                        "AllGather", OP.bypass,
                        ins=[kcon[:].opt()],
                        outs=[kout[:].opt()],
                        replica_groups=PAIRS,
                    )
                    krs = kout[bass.ds(pv, 1)]
                    for e in range(DT):
                        nc.sync.dma_start(KT[e][:, T : 2 * T],
                                          krs[0, :, e * T : (e + 1) * T])

                    wv_sb = wpool.tile([P, DT * D], F16, tag="w", name="wv_sb")
                    nc.sync.dma_start(wv_sb[:], wvT[l])
                    for m in range(4):
                        for c in range(2):
                            v_ps = psr.tile([P, 6 * DH], F32, tag="rot", space="PSUM",
                                            name="v_ps")
                            for k in range(DT):
                                nc.tensor.matmul(
                                    v_ps[:], hT[k][:, m * P : (m + 1) * P],
                                    wv_sb[:, k * D + c * 6 * DH : k * D + (c + 1) * 6 * DH],
                                    start=(k == 0), stop=False)
                            nc.tensor.matmul(
                                v_ps[:], ones_row_h[:],
                                vb_sb[:, c * 6 * DH : (c + 1) * 6 * DH],
                                start=False, stop=True, skip_group_check=True)
                            dst = VT[m][:, c * 6 * (DH + 1) : (c + 1) * 6 * (DH + 1)] \
                                .rearrange("p (h e) -> p h e", h=6, e=DH + 1)[:, :, 0:DH]
                            src = v_ps[:].rearrange("p (h e) -> p h e", h=6, e=DH)
                            nc.vector.tensor_copy(dst, src)

                    # ---- share V with the pair core (second pair AllGather)
                    vcon = dpool.tile([P, 4 * VW], F16, tag="vcon", name="vcon")
                    for m in range(4):
                        nc.sync.dma_start(vcon[:, m * VW : (m + 1) * VW], VT[m][:])
                    vout = dpool.tile([2, P, 4 * VW], F16, tag="vout", name="vout")
                    nc.gpsimd.collective_compute(
                        "AllGather", OP.bypass,
                        ins=[vcon[:].opt()],
                        outs=[vout[:].opt()],
                        replica_groups=PAIRS,
                    )
                    vrs = vout[bass.ds(pv, 1)]
                    for m in range(4):
                        nc.sync.dma_start(VT[4 + m][:],
                                          vrs[0, :, m * VW : (m + 1) * VW])
                        nc.vector.tensor_scalar_mul(VT[4 + m][:], VT[4 + m][:],
                                                    remw_sb[:, 0:1])

                    wq_sb = wpool.tile([P, DT * D], F16, tag="w", name="wq_sb")
                    nc.sync.dma_start(wq_sb[:], wqT[l])
                    QT = []
                    for m in range(DT):
                        q_ps = psr.tile([P, T], F32, tag="rot", space="PSUM", name="q_ps")
                        for k in range(DT):
                            nc.tensor.matmul(
                                q_ps[:, 0:QW],
                                wq_sb[:, k * D + m * P : k * D + (m + 1) * P],
                                hT[k][:, q0:T], start=(k == 0), stop=(k == DT - 1))
                        qt = qpool.tile([P, T], F16, tag="qt", name="qt")
                        nc.scalar.activation(qt[:, 0:QW], q_ps[:, 0:QW], AF.Identity,
                                             bias=bqk[:, m : m + 1])
                        QT.append(qt)

                    # ---- attention (heads in groups of 4; own slots first)
                    # own slots: only queries >= slot start (suffix);
                    # remote slots: full width, V already zeroed on h=0 cores.
                    # QT/attnC hold queries [q0:T] at local cols [0:QW].
                    attnC = [apool.tile([P, T], F16, tag="attnC", name=f"attnC{e}")
                             for e in range(DT)]
                    for hg in range(0, NH, 4):
                        attn_ps = {}
                        for h in range(hg, hg + 4):
                            attn_ps[h] = psm.tile([DH + 1, T], F32, tag="acc",
                                                  space="PSUM", name=f"attnps{h}")
                        for j in range(KB):
                            c0g = max(j * P, q0) if j < 4 else q0
                            ql = c0g - q0
                            N = T - c0g
                            es = {}
                            for h in range(hg, hg + 4):
                                et, base = h // 2, (h % 2) * DH
                                pp = psr if h % 2 == 0 else psm
                                s_ps = pp.tile([P, T], F32,
                                               tag="rot" if h % 2 == 0 else "acc",
                                               space="PSUM", name="s_ps")
                                nc.tensor.matmul(
                                    s_ps[:, 0:N],
                                    KT[et][base : base + DH, j * P : (j + 1) * P],
                                    QT[et][base : base + DH, ql : ql + N],
                                    start=True, stop=True)
                                e_sb = epool.tile([P, T], F16, tag="e", name="e_sb")
                                nc.scalar.activation(e_sb[:, 0:N], s_ps[:, 0:N],
                                                     AF.Exp, scale=SCALE)
                                if j < 4 and j * P >= q0:
                                    nc.vector.tensor_mul(e_sb[:, 0:P], e_sb[:, 0:P],
                                                         trimask[:])
                                es[h] = e_sb
                            if j == KB - 1:
                                den4 = rpool.tile([1, 4 * T], F32, tag="recip",
                                                  name="den4")
                            for h in range(hg, hg + 4):
                                nc.tensor.matmul(
                                    attn_ps[h][:, ql : ql + N],
                                    VT[j][:, h * (DH + 1) : (h + 1) * (DH + 1)],
                                    es[h][:, 0:N],
                                    start=(j == 0), stop=(j == KB - 1))
                                if j == KB - 1:
                                    i = h - hg
                                    nc.vector.tensor_copy(
                                        den4[0:1, i * QW : i * QW + QW],
                                        attn_ps[h][DH : DH + 1, 0:QW])
                        rec4 = rpool.tile([1, 4 * T], F32, tag="recip", name="rec4")
                        nc.vector.reciprocal_approx_fast(out=rec4[:, 0 : 4 * QW],
                                                         in_=den4[:, 0 : 4 * QW])
                        rec4r = rpool.tile([1, 4 * T], F32R, tag="recip", name="rec4r")
                        nc.vector.tensor_copy(rec4r[:, 0 : 4 * QW], rec4[:, 0 : 4 * QW])
                        for h in range(hg, hg + 4):
                            i = h - hg
                            et, base = h // 2, (h % 2) * DH
                            rb_ps = psr.tile([DH, T], F32, tag="rot", space="PSUM",
                                             name="rb_ps2")
                            nc.tensor.matmul(rb_ps[:, 0:QW], ones_row[:, 0:DH],
                                             rec4r[0:1, i * QW : i * QW + QW],
                                             start=True, stop=True)
                            rbs = epool.tile([DH, T], F32, tag="rbs", name="rbs")
                            nc.scalar.copy(rbs[:, 0:QW], rb_ps[:, 0:QW])
                            nc.vector.tensor_mul(
                                attnC[et][base : base + DH, 0:QW],
                                attn_ps[h][0:DH, 0:QW], rbs[:, 0:QW])

                    # ---- output projection + residual (in place)
                    wo_sb = wpool.tile([P, DT * D], F16, tag="w", name="wo_sb")
                    nc.sync.dma_start(wo_sb[:], woT[l])
                    for m in range(DT):
                        o_ps = psr.tile([P, T], F32, tag="rot", space="PSUM", name="o_ps")
                        for k in range(DT):
                            nc.tensor.matmul(
                                o_ps[:, 0:QW],
                                wo_sb[:, k * D + m * P : k * D + (m + 1) * P],
                                attnC[k][:, 0:QW], start=(k == 0), stop=(k == DT - 1))
                        nc.vector.tensor_add(xT[m][:, q0:T], o_ps[:, 0:QW],
                                             xT[m][:, q0:T])

                    # ---- FFN (gelu fused on the scalar engine, fc1-bias via
                    # the activation bias column)
                    h2T = layer_norm(l, lo=q0)
                    x2_ps = [psm.tile([P, T], F32, tag="acc", space="PSUM",
                                      name=f"x2ps{m}") for m in range(DT)]
                    for f in range(FT):
                        f1w = fpool.tile([P, DT * P], F16, tag="f1w", name="f1w")
                        nc.sync.dma_start(f1w[:], fc1T[l, f])
                        f1_ps = psr.tile([P, T], F32, tag="rot", space="PSUM", name="f1_ps")
                        for k in range(DT):
                            nc.tensor.matmul(f1_ps[:, 0:QW], f1w[:, k * P : (k + 1) * P],
                                             h2T[k][:, 0:QW],
                                             start=(k == 0), stop=(k == DT - 1))
                        f2w = fpool.tile([P, D], F16, tag="f2w", name="f2w")
                        nc.scalar.dma_start(f2w[:], fc2T[l, f])
                        g_sb = epool.tile([P, T], F16, tag="e", name="g_sb")
                        nc.scalar.activation(g_sb[:, 0:QW], f1_ps[:, 0:QW],
                                             AF.Gelu_apprx_tanh,
                                             bias=fb_sb[:, f : f + 1])
                        for m in range(DT):
                            nc.tensor.matmul(x2_ps[m][:, 0:QW],
                                             f2w[:, m * P : (m + 1) * P],
                                             g_sb[:, 0:QW],
                                             start=(f == 0), stop=(f == FT - 1))
                    for m in range(DT):
                        nc.vector.tensor_add(xT[m][:, q0:T], x2_ps[m][:, 0:QW],
                                             xT[m][:, q0:T])

            # ---- final: masked AllReduce of predicted token's x column
            with nc.named_scope("final"):
                # unembed tiles: prefetch via the (otherwise idle) GpSimd
                # queue — Sync-queue issue would serialize behind the
                # AllReduce-dependent reads. Only the first 28 fit in SBUF
                # pre-AR; the rest are issued after the AR trigger so the
                # ring-wait cannot dead-block the trigger itself.
                ut_tiles = []
                for ci in range(VCH):
                    for k in range(DT):
                        u_sb = upool.tile([P, VCW], F16, tag="ut", name="u_sb")
                        ut_tiles.append((ci, k, u_sb))
                for ci, k, u_sb in ut_tiles[:28]:
                    nc.gpsimd.dma_start(u_sb[:], uT[k, :, ci * VCW : (ci + 1) * VCW])
                cont = dpool.tile([P, DT * B], F32, tag="cont", name="cont")
                csb = spool.tile([P, DT * B], F32, tag="csb", name="csb")
                for k in range(DT):
                    nc.vector.tensor_mul(
                        csb[:, k * B : (k + 1) * B],
                        xT[k][:, pcol : pcol + 1].to_broadcast((P, B)),
                        sel4_sb[:])
                nc.sync.dma_start(cont[:], csb[:])
                ar_out = dpool.tile([P, DT * B], F32, tag="arout",
                                    addr_space="Shared", name="ar_out")
                nc.gpsimd.collective_compute(
                    "AllReduce", OP.add,
                    ins=[cont[:].opt()],
                    outs=[ar_out[:].opt()],
                    replica_groups=[list(range(NC_))],
                )
                for ci, k, u_sb in ut_tiles[28:]:
                    nc.gpsimd.dma_start(u_sb[:], uT[k, :, ci * VCW : (ci + 1) * VCW])
                xf_raw = spool.tile([P, DT * B], F32, tag="xfraw", name="xf_raw")
                nc.sync.dma_start(xf_raw[:], ar_out[:])
                xf = spool.tile([P, DT * B], F32R, tag="xf", name="xf")
                nc.vector.tensor_copy(xf[:], xf_raw[:])

                fs_ps = psm.tile([1, B], F32, tag="acc", space="PSUM", name="fs_ps")
                fq_ps = psm.tile([1, B], F32, tag="acc", space="PSUM", name="fq_ps")
                xfsq = spool.tile([P, DT * B], F32R, tag="xfsq", name="xfsq")
                nc.vector.tensor_mul(xfsq[:], xf[:], xf[:])
                for k in range(DT):
                    nc.tensor.matmul(fs_ps[:], ones_col[:], xf[:, k * B : (k + 1) * B],
                                     start=(k == 0), stop=(k == DT - 1))
                    nc.tensor.matmul(fq_ps[:], ones_col[:], xfsq[:, k * B : (k + 1) * B],
                                     start=(k == 0), stop=(k == DT - 1))
                fmean = spool.tile([1, B], F32, tag="lnstat", name="fmean")
                nc.vector.tensor_scalar_mul(fmean[:], fs_ps[:], 1.0 / D)
                fm2 = spool.tile([1, B], F32, tag="lnstat", name="fm2")
                nc.vector.tensor_mul(fm2[:], fmean[:], fmean[:])
                fsqd = spool.tile([1, B], F32, tag="lnstat", name="fsqd")
                nc.vector.tensor_scalar_mul(fsqd[:], fq_ps[:], 1.0 / D)
                fvar = spool.tile([1, B], F32, tag="lnstat", name="fvar")
                nc.vector.tensor_sub(fvar[:], fsqd[:], fm2[:])
                fstd = spool.tile([1, B], F32, tag="lnstat", name="fstd")
                nc.scalar.activation(fstd[:], fvar[:], AF.Sqrt, bias=eps1[:])
                frstd = spool.tile([1, B], F32, tag="lnr", name="frstd")
                nc.vector.reciprocal(frstd[:], fstd[:])
                frstd_r = spool.tile([1, B], F32R, tag="lnr", name="frstd_r")
                nc.vector.tensor_copy(frstd_r[:], frstd[:])
                fmrs = spool.tile([1, B], F32R, tag="lnr", name="fmrs")
                nc.vector.tensor_mul(fmrs[:], fmean[:], frstd[:])
                fr_ps = psr.tile([P, B], F32, tag="rot", space="PSUM", name="fr_ps")
                nc.tensor.matmul(fr_ps[:], ones_row[:], frstd_r[:],
                                 start=True, stop=True)
                fm_ps = psr.tile([P, B], F32, tag="rot", space="PSUM", name="fm_ps")
                nc.tensor.matmul(fm_ps[:], ones_row[:], fmrs[:],
                                 start=True, stop=True)
                xfn = spool.tile([P, DT * B], F16, tag="xfn", name="xfn")
                for k in range(DT):
                    t1 = spool.tile([P, B], F32, tag="lnstat", name="ft1")
                    nc.vector.tensor_mul(t1[:], xf[:, k * B : (k + 1) * B], fr_ps[:])
                    nc.vector.tensor_sub(xfn[:, k * B : (k + 1) * B], t1[:], fm_ps[:])

                for ci in range(VCH):
                    lg_ps = psr.tile([B, VCW], F32, tag="rot", space="PSUM", name="lg_ps")
                    for k in range(DT):
                        u_sb = ut_tiles[ci * DT + k][2]
                        nc.tensor.matmul(lg_ps[:], xfn[:, k * B : (k + 1) * B], u_sb[:],
                                         start=(k == 0), stop=(k == DT - 1))
                    och = fpool.tile([B, VCW], F32, tag="f2w", name="och")
                    nc.vector.tensor_copy(och[:], lg_ps[:])
                    nc.sync.dma_start(out[:, ci * VCW : (ci + 1) * VCW], och[:])

    nc.compile()
    return nc


# ---------------------------------------------------------------- host side
def _positional_encoding(s, d):
    idx = np.arange(d)
    exponent = ((2 * (idx // 2)).astype(np.float32) / float(d)).astype(np.float32)
    pos = np.arange(s, dtype=np.float32)[:, None]
    angle = pos / np.power(np.float32(10000.0), exponent[None, :], dtype=np.float32)
    return np.where((idx % 2 == 0)[None, :], np.sin(angle), np.cos(angle)).astype(np.float32)


def _build_masks():
    """trimask[r, c] = 1 if key r <= query c (within-block causal)."""
    r = np.arange(P)
    return (r[:, None] <= r[None, :]).astype(np.float16)


def prepare_inputs(tokens, predict_idx, embedding, ln1_g, ln1_b, wq, wk, wv, wo,
                   ln2_g, ln2_b, fc1, fc2, lnf_g, lnf_b, unembed, n_layers=NL):
    f = lambda a: np.ascontiguousarray(np.asarray(a), dtype=np.float32)
    tokens = np.asarray(tokens)
    emb = f(embedding)
    pos = _positional_encoding(S, D)

    ln1_g, ln1_b = f(ln1_g)[:n_layers], f(ln1_b)[:n_layers]
    ln2_g, ln2_b = f(ln2_g)[:n_layers], f(ln2_b)[:n_layers]
    lnf_gf, lnf_bf = f(lnf_g), f(lnf_b)

    def wlayout(a):  # [L, out, in] -> [L, P, DT*D] with [l, p, k*D + dout]
        aT = a.transpose(0, 2, 1)
        return np.ascontiguousarray(
            aT.reshape(n_layers, DT, P, D).transpose(0, 2, 1, 3)
            .reshape(n_layers, P, DT * D)).astype(np.float16)

    wq2 = f(wq)[:n_layers].reshape(-1, NH * DH, D)
    wk2 = f(wk)[:n_layers].reshape(-1, NH * DH, D)
    wv2 = f(wv)[:n_layers].reshape(-1, NH * DH, D)
    # fold LN1 gain into the QKV weight columns; betas become bias vectors
    wqT = wlayout(wq2 * ln1_g[:, None, :])
    wkT = wlayout(wk2 * ln1_g[:, None, :])
    wvT = wlayout(wv2 * ln1_g[:, None, :])
    woT = wlayout(f(wo)[:n_layers])
    cbq = np.einsum("lod,ld->lo", wq2, ln1_b)   # [L, 768]
    cbk = np.einsum("lod,ld->lo", wk2, ln1_b)
    cbv = np.einsum("lod,ld->lo", wv2, ln1_b)
    qkbias = np.stack([
        np.concatenate([cbq[l].reshape(DT, P).T, cbk[l].reshape(DT, P).T], axis=1)
        for l in range(n_layers)]).astype(np.float32)  # [L, P, 2*DT]
    vbias = cbv.reshape(n_layers, 1, NH * DH).astype(np.float16)

    fc1f = f(fc1)[:n_layers]
    fc2f = f(fc2)[:n_layers]
    fc1g = fc1f * ln2_g[:, None, :]
    fc1T = np.ascontiguousarray(
        fc1g.transpose(0, 2, 1)
        .reshape(n_layers, DT, P, FT, P).transpose(0, 3, 2, 1, 4)
        .reshape(n_layers, FT, P, DT * P)).astype(np.float16)
    fc2T = np.ascontiguousarray(
        fc2f.transpose(0, 2, 1)
        .reshape(n_layers, FT, P, D)).astype(np.float16)
    cbf = np.einsum("lod,ld->lo", fc1f, ln2_b)  # [L, 3072]
    fbias = np.ascontiguousarray(
        cbf.reshape(n_layers, FT, P).transpose(0, 2, 1)).astype(np.float32)

    uf = f(unembed) * lnf_gf[None, :]
    uTf = np.ascontiguousarray(uf.T.reshape(DT, P, V)).astype(np.float16)
    ubias = f(unembed) @ lnf_bf  # [V], added host-side

    bc2 = np.zeros((2, P), np.float32)
    bc2[0, :DH] = 1.0
    bc2[1, DH:] = 1.0

    masks = _build_masks()

    pidx = int(predict_idx)
    in_maps = []
    for c in range(NC_):
        b, h = c // 2, c % 2
        toks = np.asarray(tokens[b, h * T : (h + 1) * T]).astype(np.int64)
        x0 = emb.T[toks] + pos[h * T : (h + 1) * T]
        x0T = np.ascontiguousarray(x0.T.reshape(DT, P, T)).astype(np.float32)
        sel4 = np.zeros((P, B), np.float32)
        if pidx // T == h:
            sel4[:, b] = 1.0
        m = {
            "x0T": x0T, "wqT": wqT, "wkT": wkT, "wvT": wvT, "woT": woT,
            "fc1T": fc1T, "fc2T": fc2T,
            "qkb": qkbias, "vb": vbias, "fb": fbias,
            "uT": uTf[:, :, c * VS : (c + 1) * VS].copy(),
            "masks": masks,
            "remw": np.full((P, 1), 1.0 if h == 1 else 0.0, np.float32),
            "sel4": sel4,
            "pairsel": np.array([[1 - h]], np.int32),
            "bc2d": bc2,
        }
        in_maps.append(m)
    return in_maps, ubias


_CACHED = {}


def kernel(**inputs):
    from concourse.bass_utils import run_bass_kernel_spmd
    pidx = int(np.asarray(inputs["predict_idx"]))
    key = ("nc", pidx % T)
    if key not in _CACHED:
        _CACHED[key] = build_nc(pcol=pidx % T)
    nc = _CACHED[key]
    in_maps, ubias = prepare_inputs(**inputs)
    res = run_bass_kernel_spmd(nc, in_maps, core_ids=list(range(NC_)), trace=False)
    full = np.concatenate([res.results[c]["out"] for c in range(NC_)], axis=1)
    return full + ubias[None, :]


# revision 38
# speedup vs baseline: 1.0954x; 1.0954x over previous
"""MicroGPT forward pass on 8 Trainium2 NeuronCores (Bass/Tile).

Sharding: token-sharded — core c = 2*b + h owns batch b, sequence half h
(512 contiguous tokens). Activations are feature-major in SBUF
(x^T: [768 rows -> 6 tiles of 128, 512 token cols]); all matmuls fp32r/f16.
LN gains are folded into the consumer weights host-side; LN betas become
bias columns added at PSUM eviction (Q/K via activation bias, V via an
extra ones-row matmul, fc1 via the gelu activation bias, unembed host-side).
Per-token LN stats (1/std, mean/std) are broadcast across partitions with
ones-row matmuls into PSUM (no DRAM roundtrips); softmax reciprocals
likewise via a 2-row selector matmul per head pair.
Attention: S^T = K^T-slice (stationary) x Q^T (moving); softmax without max
subtraction (scores bounded); denominators via a ones column appended to V.
K/V slots 0-3 are the core's own 4 blocks (block-causal masks on GpSimd);
slots 4-7 are the pair core's blocks, weighted by a per-core 0/1 scalar.
Per layer a PAIR-group AllGather ([[0,1],[2,3],[4,5],[6,7]]) shares K^T/V;
warmup collectives at program start absorb the NRT rendezvous cost.
Final token: masked 8-way AllReduce, then final LN + vocab-sharded unembed
(4000 vocab rows per core); unembed bias applied host-side.
"""
import sys, math

sys.path.insert(0, "/opt/trn_rl_repo")
import numpy as np

import concourse.bass as bass
import concourse.bacc as bacc
import concourse.mybir as mybir
import concourse.tile as tile

D, NH, DH, FF, NL, V = 768, 12, 64, 3072, 4, 32000
B, S = 4, 1024
EPS = 1e-5
NC_ = 8
P = 128
T = 512            # tokens per core
DT = D // P        # 6 d-model tiles
FT = FF // P       # 24 ff tiles
KB = 8             # key slots (0-3 own, 4-7 pair)
VS = V // NC_      # 4000 vocab rows per core
VCH = 8            # vocab chunks of 500
VCW = VS // VCH    # 500
F32 = mybir.dt.float32
F32R = mybir.dt.float32r
F16 = mybir.dt.float16
BF16 = mybir.dt.bfloat16
I32 = mybir.dt.int32
AF = mybir.ActivationFunctionType
OP = mybir.AluOpType
SCALE = 1.0 / math.sqrt(DH)
VW = NH * (DH + 1)           # 780 — V tile width incl. ones cols
CONTRIB_W = DT * T + 4 * VW  # AllGather contribution width
PAIRS = [[0, 1], [2, 3], [4, 5], [6, 7]]


# ---------------------------------------------------------------- bass program
def build_nc(n_layers=NL, pcol=511, dbg=False):
    nc = bacc.Bacc(None, target_bir_lowering=False, debug=False, num_devices=NC_)

    x0T = nc.dram_tensor("x0T", [DT, P, T], F32R, kind="ExternalInput")
    wqT = nc.dram_tensor("wqT", [n_layers, P, DT * D], F16, kind="ExternalInput")
    wkT = nc.dram_tensor("wkT", [n_layers, P, DT * D], F16, kind="ExternalInput")
    wvT = nc.dram_tensor("wvT", [n_layers, P, DT * D], F16, kind="ExternalInput")
    woT = nc.dram_tensor("woT", [n_layers, P, DT * D], F16, kind="ExternalInput")
    fc1T = nc.dram_tensor("fc1T", [n_layers, FT, P, DT * P], F16, kind="ExternalInput")
    fc2T = nc.dram_tensor("fc2T", [n_layers, FT, P, D], F16, kind="ExternalInput")
    qkb = nc.dram_tensor("qkb", [n_layers, P, 2 * DT], F32, kind="ExternalInput")
    vb = nc.dram_tensor("vb", [n_layers, 1, NH * DH], F16, kind="ExternalInput")
    fb = nc.dram_tensor("fb", [n_layers, P, FT], F32, kind="ExternalInput")
    uT = nc.dram_tensor("uT", [DT, P, VS], F16, kind="ExternalInput")
    masks = nc.dram_tensor("masks", [P, P], F16, kind="ExternalInput")
    remw = nc.dram_tensor("remw", [P, 1], F32, kind="ExternalInput")
    sel4 = nc.dram_tensor("sel4", [P, B], F32R, kind="ExternalInput")
    pairsel = nc.dram_tensor("pairsel", [1, 1], I32, kind="ExternalInput")
    bc2d = nc.dram_tensor("bc2d", [2, P], F32R, kind="ExternalInput")

    out = nc.dram_tensor("out", [B, VS], F32, kind="ExternalOutput")

    with tile.TileContext(nc) as tc:
        with (
            tc.tile_pool(name="const", bufs=1) as cpool,
            tc.tile_pool(name="persist", bufs=1) as ppool,
            tc.tile_pool(name="xp", bufs=6) as xpool,
            tc.tile_pool(name="hp", bufs=7) as hpool,
            tc.tile_pool(name="qp", bufs=6) as qpool,
            tc.tile_pool(name="ac", bufs=6) as apool,
            tc.tile_pool(name="wp", bufs=2) as wpool,
            tc.tile_pool(name="fp", bufs=3) as fpool,
            tc.tile_pool(name="ep", bufs=5) as epool,
            tc.tile_pool(name="sp", bufs=5) as spool,
            tc.tile_pool(name="up", bufs=28) as upool,
            tc.tile_pool(name="rp", bufs=3) as rpool,
            tc.tile_pool(name="psm", bufs=6, space="PSUM") as psm,
            tc.tile_pool(name="psr", bufs=2, space="PSUM") as psr,
            tc.tile_pool(name="dram", bufs=2, space="DRAM") as dpool,
        ):
            # pair rank register for dynamic reads of the AllGather output.
            # MUST precede the warmup collectives: the critical section
            # serializes the Sync engine (which issues all DMAs) against all
            # prior work, so putting it after the warmups stalls every input
            # DMA until the collective rendezvous completes (~50us).
            with tc.tile_critical():
                with nc.sync.register("pairreg") as preg:
                    nc.sync.reg_load(preg, pairsel[0:1, 0:1])
                    pv = nc.sync.snap(preg, min_val=0, max_val=1)

            # ---- warmup collectives: absorb NRT rendezvous cost while the
            # input DMAs + L0 LN/KV run. Same shapes of groups as the real ops.
            wg_in = dpool.tile([P, 8], F16, tag="wua", name="wg_in")
            wg_out = dpool.tile([2, P, 8], F16, tag="wub", name="wg_out")
            nc.gpsimd.collective_compute(
                "AllGather", OP.bypass,
                ins=[wg_in[:].opt()], outs=[wg_out[:].opt()],
                replica_groups=PAIRS,
            )
            # (the AllReduce warmup is issued later, after L0's V AllGather:
            # issued here its trigger head-blocks the GpSimd queue for ~50us,
            # delaying L0's K AllGather; dropping it entirely slows every
            # later collective — it must run once, just not first)
            wr_in = dpool.tile([P, 8], F32, tag="wuc", name="wr_in")
            wr_out = dpool.tile([P, 8], F32, tag="wud", addr_space="Shared",
                                name="wr_out")

            # ---- constants (memset cannot write f32r; stage via f32 + copy)
            ones_f32 = cpool.tile([P, 1], F32)
            nc.vector.memset(ones_f32[:], 1.0)
            trimask = cpool.tile([P, P], F16)
            nc.sync.dma_start(trimask[:], masks[:])
            ones_col = cpool.tile([P, 1], F32R)
            nc.vector.tensor_copy(ones_col[:], ones_f32[:])
            onesr_f32 = cpool.tile([1, P], F32)
            nc.vector.memset(onesr_f32[:], 1.0)
            ones_row = cpool.tile([1, P], F32R)
            nc.vector.tensor_copy(ones_row[:], onesr_f32[:])
            ones_row_h = cpool.tile([1, P], F16)
            nc.vector.tensor_copy(ones_row_h[:], onesr_f32[:])
            bc2 = cpool.tile([2, P], F32R)
            nc.sync.dma_start(bc2[:], bc2d[:])
            eps1 = cpool.tile([1, 1], F32)
            nc.vector.memset(eps1[:], EPS)
            sel4_sb = cpool.tile([P, B], F32R)
            nc.sync.dma_start(sel4_sb[:], sel4[:])
            remw_sb = cpool.tile([P, 1], F32)
            nc.sync.dma_start(remw_sb[:], remw[:])

            # persistent K^T / V buffers (slots 0-3 own, 4-7 pair)
            KT = [ppool.tile([P, KB * P], F16, tag=f"kt{e}", name=f"KT{e}")
                  for e in range(DT)]
            VT = [ppool.tile([P, VW], F16, tag=f"vt{j}", name=f"VT{j}")
                  for j in range(KB)]
            for j in range(4):
                for h in range(NH):
                    nc.vector.tensor_copy(
                        VT[j][:, h * (DH + 1) + DH : h * (DH + 1) + DH + 1],
                        ones_f32[:])

            # ---- residual stream (updated in place by residual adds)
            xT = []
            for k in range(DT):
                t_ = xpool.tile([P, T], F32R, tag="xT", name=f"xT{k}")
                nc.sync.dma_start(t_[:], x0T[k])
                xT.append(t_)

            def layer_norm(l, lo=0):
                """Stats via ones-column matmuls; rstd/mrs broadcast across
                partitions via ones-row matmuls into PSUM. Returns y tiles
                (normalized WITHOUT gain/bias — those are folded into the
                consumer weights). With lo>0, only token columns [lo:T] are
                processed; returned tiles hold them at local cols [0:T-lo]."""
                W = T - lo
                sum_ps = psm.tile([1, T], F32, tag="acc", space="PSUM", name="sum_ps")
                sq_ps = psm.tile([1, T], F32, tag="acc", space="PSUM", name="sq_ps")
                for k in range(DT):
                    xsq = epool.tile([P, T], F32R, tag="lntmp", name="xsq")
                    nc.vector.tensor_mul(xsq[:, 0:W], xT[k][:, lo:T], xT[k][:, lo:T])
                    nc.tensor.matmul(sum_ps[:, 0:W], ones_col[:], xT[k][:, lo:T],
                                     start=(k == 0), stop=(k == DT - 1))
                    nc.tensor.matmul(sq_ps[:, 0:W], ones_col[:], xsq[:, 0:W],
                                     start=(k == 0), stop=(k == DT - 1))
                sums_sb = spool.tile([1, T], F32, tag="lnstat", name="sums_sb")
                nc.vector.tensor_copy(sums_sb[:, 0:W], sum_ps[:, 0:W])
                m2s = spool.tile([1, T], F32, tag="lnstat", name="m2s")
                nc.vector.scalar_tensor_tensor(out=m2s[:, 0:W], in0=sums_sb[:, 0:W],
                                               scalar=1.0 / (D * D), in1=sums_sb[:, 0:W],
                                               op0=OP.mult, op1=OP.mult)
                var = spool.tile([1, T], F32, tag="lnstat", name="var")
                nc.vector.scalar_tensor_tensor(out=var[:, 0:W], in0=sq_ps[:, 0:W],
                                               scalar=1.0 / D, in1=m2s[:, 0:W],
                                               op0=OP.mult, op1=OP.subtract)
                rstd = spool.tile([1, T], F32, tag="lnr", name="rstd")
                nc.scalar.activation(rstd[:, 0:W], var[:, 0:W],
                                     AF.Abs_reciprocal_sqrt, bias=eps1[:])
                rstd_r = spool.tile([1, T], F32R, tag="lnr", name="rstd_r")
                nc.vector.tensor_copy(rstd_r[:, 0:W], rstd[:, 0:W])
                mrs = spool.tile([1, T], F32R, tag="lnr", name="mrs")
                nc.vector.scalar_tensor_tensor(out=mrs[:, 0:W], in0=sums_sb[:, 0:W],
                                               scalar=1.0 / D, in1=rstd[:, 0:W],
                                               op0=OP.mult, op1=OP.mult)
                rb_ps = psr.tile([P, T], F32, tag="rot", space="PSUM", name="rb_ps")
                nc.tensor.matmul(rb_ps[:, 0:W], ones_row[:], rstd_r[:, 0:W],
                                 start=True, stop=True)
                mb_ps = psr.tile([P, T], F32, tag="rot", space="PSUM", name="mb_ps")
                nc.tensor.matmul(mb_ps[:, 0:W], ones_row[:], mrs[:, 0:W],
                                 start=True, stop=True)
                hT = []
                for k in range(DT):
                    t1 = epool.tile([P, T], F32R, tag="lntmp", name="lnt1")
                    nc.vector.tensor_mul(t1[:, 0:W], xT[k][:, lo:T], rb_ps[:, 0:W])
                    h_ = hpool.tile([P, T], F16, tag="hT", name="hT_t")
                    nc.vector.tensor_sub(h_[:, 0:W], t1[:, 0:W], mb_ps[:, 0:W])
                    hT.append(h_)
                return hT

            for l in range(n_layers):
                with nc.named_scope(f"L{l}"):
                    # Last layer: only the 128-col query block holding pcol
                    # feeds the output — restrict all query-side work to it.
                    # (K/V stay full width: every key is still attended.)
                    q0 = (pcol // P) * P if l == n_layers - 1 else 0
                    QW = T - q0
                    bqk = spool.tile([P, 2 * DT], F32, tag="lngb", name="bqk")
                    nc.sync.dma_start(bqk[:], qkb[l])
                    vb_sb = spool.tile([1, NH * DH], F16, tag="vbr", name="vb_sb")
                    nc.sync.dma_start(vb_sb[:], vb[l])
                    fb_sb = spool.tile([P, FT], F32, tag="fbg", name="fb_sb")
                    nc.sync.dma_start(fb_sb[:], fb[l])

                    hT = layer_norm(l)

                    # ---- K^T first: its pair AllGather launches while V/Q
                    # are still computing (remote K is needed first, by QK)
                    wk_sb = wpool.tile([P, DT * D], F16, tag="w", name="wk_sb")
                    nc.sync.dma_start(wk_sb[:], wkT[l])
                    for m in range(DT):
                        k_ps = psr.tile([P, T], F32, tag="rot", space="PSUM", name="k_ps")
                        for k in range(DT):
                            nc.tensor.matmul(
                                k_ps[:], wk_sb[:, k * D + m * P : k * D + (m + 1) * P],
                                hT[k][:], start=(k == 0), stop=(k == DT - 1))
                        nc.scalar.activation(KT[m][:, 0:T], k_ps[:], AF.Identity,
                                             bias=bqk[:, DT + m : DT + m + 1])
                    kcon = dpool.tile([P, DT * T], F16, tag="kcon", name="kcon")
                    for e in range(DT):
                        nc.sync.dma_start(kcon[:, e * T : (e + 1) * T],
                                          KT[e][:, 0:T])
                    kout = dpool.tile([2, P, DT * T], F16, tag="kout", name="kout")
                    nc.gpsimd.collective_compute(
                        "AllGather", OP.bypass,
                        ins=[kcon[:].opt()],
                        outs=[kout[:].opt()],
                        replica_groups=PAIRS,
                    )
                    krs = kout[bass.ds(pv, 1)]
                    for e in range(DT):
                        nc.sync.dma_start(KT[e][:, T : 2 * T],
                                          krs[0, :, e * T : (e + 1) * T])

                    wv_sb = wpool.tile([P, DT * D], F16, tag="w", name="wv_sb")
                    nc.sync.dma_start(wv_sb[:], wvT[l])
                    for m in range(4):
                        for c in range(2):
                            v_ps = psr.tile([P, 6 * DH], F32, tag="rot", space="PSUM",
                                            name="v_ps")
                            for k in range(DT):
                                nc.tensor.matmul(
                                    v_ps[:], hT[k][:, m * P : (m + 1) * P],
                                    wv_sb[:, k * D + c * 6 * DH : k * D + (c + 1) * 6 * DH],
                                    start=(k == 0), stop=False)
                            nc.tensor.matmul(
                                v_ps[:], ones_row_h[:],
                                vb_sb[:, c * 6 * DH : (c + 1) * 6 * DH],
                                start=False, stop=True, skip_group_check=True)
                            dst = VT[m][:, c * 6 * (DH + 1) : (c + 1) * 6 * (DH + 1)] \
                                .rearrange("p (h e) -> p h e", h=6, e=DH + 1)[:, :, 0:DH]
                            src = v_ps[:].rearrange("p (h e) -> p h e", h=6, e=DH)
                            nc.vector.tensor_copy(dst, src)

                    # ---- share V with the pair core (second pair AllGather)
                    vcon = dpool.tile([P, 4 * VW], F16, tag="vcon", name="vcon")
                    for m in range(4):
                        nc.sync.dma_start(vcon[:, m * VW : (m + 1) * VW], VT[m][:])
                    vout = dpool.tile([2, P, 4 * VW], F16, tag="vout", name="vout")
                    nc.gpsimd.collective_compute(
                        "AllGather", OP.bypass,
                        ins=[vcon[:].opt()],
                        outs=[vout[:].opt()],
                        replica_groups=PAIRS,
                    )
                    vrs = vout[bass.ds(pv, 1)]
                    for m in range(4):
                        nc.sync.dma_start(VT[4 + m][:],
                                          vrs[0, :, m * VW : (m + 1) * VW])
                        nc.vector.tensor_scalar_mul(VT[4 + m][:], VT[4 + m][:],
                                                    remw_sb[:, 0:1])
                    if l == 0:
                        # 8-way AllReduce warmup (see note at the warmups)
                        nc.gpsimd.collective_compute(
                            "AllReduce", OP.add,
                            ins=[wr_in[:].opt()], outs=[wr_out[:].opt()],
                            replica_groups=[list(range(NC_))],
                        )

                    wq_sb = wpool.tile([P, DT * D], F16, tag="w", name="wq_sb")
                    nc.sync.dma_start(wq_sb[:], wqT[l])
                    QT = []
                    for m in range(DT):
                        q_ps = psr.tile([P, T], F32, tag="rot", space="PSUM", name="q_ps")
                        for k in range(DT):
                            nc.tensor.matmul(
                                q_ps[:, 0:QW],
                                wq_sb[:, k * D + m * P : k * D + (m + 1) * P],
                                hT[k][:, q0:T], start=(k == 0), stop=(k == DT - 1))
                        qt = qpool.tile([P, T], F16, tag="qt", name="qt")
                        nc.scalar.activation(qt[:, 0:QW], q_ps[:, 0:QW], AF.Identity,
                                             bias=bqk[:, m : m + 1])
                        QT.append(qt)

                    # ---- attention (heads in groups of 4; own slots first)
                    # own slots: only queries >= slot start (suffix);
                    # remote slots: full width, V already zeroed on h=0 cores.
                    # QT/attnC hold queries [q0:T] at local cols [0:QW].
                    attnC = [apool.tile([P, T], F16, tag="attnC", name=f"attnC{e}")
                             for e in range(DT)]
                    for hg in range(0, NH, 4):
                        attn_ps = {}
                        for h in range(hg, hg + 4):
                            attn_ps[h] = psm.tile([DH + 1, T], F32, tag="acc",
                                                  space="PSUM", name=f"attnps{h}")
                        for j in range(KB):
                            c0g = max(j * P, q0) if j < 4 else q0
                            ql = c0g - q0
                            N = T - c0g
                            es = {}
                            for h in range(hg, hg + 4):
                                et, base = h // 2, (h % 2) * DH
                                pp = psr if h % 2 == 0 else psm
                                s_ps = pp.tile([P, T], F32,
                                               tag="rot" if h % 2 == 0 else "acc",
                                               space="PSUM", name="s_ps")
                                nc.tensor.matmul(
                                    s_ps[:, 0:N],
                                    KT[et][base : base + DH, j * P : (j + 1) * P],
                                    QT[et][base : base + DH, ql : ql + N],
                                    start=True, stop=True)
                                e_sb = epool.tile([P, T], F16, tag="e", name="e_sb")
                                nc.scalar.activation(e_sb[:, 0:N], s_ps[:, 0:N],
                                                     AF.Exp, scale=SCALE)
                                if j < 4 and j * P >= q0:
                                    nc.vector.tensor_mul(e_sb[:, 0:P], e_sb[:, 0:P],
                                                         trimask[:])
                                es[h] = e_sb
                            if j == KB - 1:
                                den4 = rpool.tile([1, 4 * T], F32, tag="recip",
                                                  name="den4")
                            for h in range(hg, hg + 4):
                                nc.tensor.matmul(
                                    attn_ps[h][:, ql : ql + N],
                                    VT[j][:, h * (DH + 1) : (h + 1) * (DH + 1)],
                                    es[h][:, 0:N],
                                    start=(j == 0), stop=(j == KB - 1))
                                if j == KB - 1:
                                    i = h - hg
                                    nc.vector.tensor_copy(
                                        den4[0:1, i * QW : i * QW + QW],
                                        attn_ps[h][DH : DH + 1, 0:QW])
                        rec4 = rpool.tile([1, 4 * T], F32, tag="recip", name="rec4")
                        nc.vector.reciprocal_approx_fast(out=rec4[:, 0 : 4 * QW],
                                                         in_=den4[:, 0 : 4 * QW])
                        rec4r = rpool.tile([1, 4 * T], F32R, tag="recip", name="rec4r")
                        nc.vector.tensor_copy(rec4r[:, 0 : 4 * QW], rec4[:, 0 : 4 * QW])
                        for h in range(hg, hg + 4):
                            i = h - hg
                            et, base = h // 2, (h % 2) * DH
                            rb_ps = psr.tile([DH, T], F32, tag="rot", space="PSUM",
                                             name="rb_ps2")
                            nc.tensor.matmul(rb_ps[:, 0:QW], ones_row[:, 0:DH],
                                             rec4r[0:1, i * QW : i * QW + QW],
                                             start=True, stop=True)
                            rbs = epool.tile([DH, T], F32, tag="rbs", name="rbs")
                            nc.scalar.copy(rbs[:, 0:QW], rb_ps[:, 0:QW])
                            nc.vector.tensor_mul(
                                attnC[et][base : base + DH, 0:QW],
                                attn_ps[h][0:DH, 0:QW], rbs[:, 0:QW])

                    # ---- output projection + residual (in place)
                    wo_sb = wpool.tile([P, DT * D], F16, tag="w", name="wo_sb")
                    nc.sync.dma_start(wo_sb[:], woT[l])
                    for m in range(DT):
                        o_ps = psr.tile([P, T], F32, tag="rot", space="PSUM", name="o_ps")
                        for k in range(DT):
                            nc.tensor.matmul(
                                o_ps[:, 0:QW],
                                wo_sb[:, k * D + m * P : k * D + (m + 1) * P],
                                attnC[k][:, 0:QW], start=(k == 0), stop=(k == DT - 1))
                        nc.vector.tensor_add(xT[m][:, q0:T], o_ps[:, 0:QW],
                                             xT[m][:, q0:T])

                    # ---- FFN (gelu fused on the scalar engine, fc1-bias via
                    # the activation bias column)
                    h2T = layer_norm(l, lo=q0)
                    x2_ps = [psm.tile([P, T], F32, tag="acc", space="PSUM",
                                      name=f"x2ps{m}") for m in range(DT)]
                    for f in range(FT):
                        f1w = fpool.tile([P, DT * P], F16, tag="f1w", name="f1w")
                        nc.sync.dma_start(f1w[:], fc1T[l, f])
                        f1_ps = psr.tile([P, T], F32, tag="rot", space="PSUM", name="f1_ps")
                        for k in range(DT):
                            nc.tensor.matmul(f1_ps[:, 0:QW], f1w[:, k * P : (k + 1) * P],
                                             h2T[k][:, 0:QW],
                                             start=(k == 0), stop=(k == DT - 1))
                        f2w = fpool.tile([P, D], F16, tag="f2w", name="f2w")
                        nc.scalar.dma_start(f2w[:], fc2T[l, f])
                        g_sb = epool.tile([P, T], F16, tag="e", name="g_sb")
                        nc.scalar.activation(g_sb[:, 0:QW], f1_ps[:, 0:QW],
                                             AF.Gelu_apprx_tanh,
                                             bias=fb_sb[:, f : f + 1])
                        for m in range(DT):
                            nc.tensor.matmul(x2_ps[m][:, 0:QW],
                                             f2w[:, m * P : (m + 1) * P],
                                             g_sb[:, 0:QW],
                                             start=(f == 0), stop=(f == FT - 1))
                    for m in range(DT):
                        nc.vector.tensor_add(xT[m][:, q0:T], x2_ps[m][:, 0:QW],
                                             xT[m][:, q0:T])

            # ---- final: masked AllReduce of predicted token's x column
            with nc.named_scope("final"):
                # unembed tiles: prefetch via the (otherwise idle) GpSimd
                # queue — Sync-queue issue would serialize behind the
                # AllReduce-dependent reads. Only the first 28 fit in SBUF
                # pre-AR; the rest are issued after the AR trigger so the
                # ring-wait cannot dead-block the trigger itself.
                ut_tiles = []
                for ci in range(VCH):
                    for k in range(DT):
                        u_sb = upool.tile([P, VCW], F16, tag="ut", name="u_sb")
                        ut_tiles.append((ci, k, u_sb))
                for ci, k, u_sb in ut_tiles[:28]:
                    nc.gpsimd.dma_start(u_sb[:], uT[k, :, ci * VCW : (ci + 1) * VCW])
                cont = dpool.tile([P, DT * B], F32, tag="cont", name="cont")
                csb = spool.tile([P, DT * B], F32, tag="csb", name="csb")
                for k in range(DT):
                    nc.vector.tensor_mul(
                        csb[:, k * B : (k + 1) * B],
                        xT[k][:, pcol : pcol + 1].to_broadcast((P, B)),
                        sel4_sb[:])
                nc.sync.dma_start(cont[:], csb[:])
                ar_out = dpool.tile([P, DT * B], F32, tag="arout",
                                    addr_space="Shared", name="ar_out")
                nc.gpsimd.collective_compute(
                    "AllReduce", OP.add,
                    ins=[cont[:].opt()],
                    outs=[ar_out[:].opt()],
                    replica_groups=[list(range(NC_))],
                )
                for ci, k, u_sb in ut_tiles[28:]:
                    nc.gpsimd.dma_start(u_sb[:], uT[k, :, ci * VCW : (ci + 1) * VCW])
                xf_raw = spool.tile([P, DT * B], F32, tag="xfraw", name="xf_raw")
                nc.sync.dma_start(xf_raw[:], ar_out[:])
                xf = spool.tile([P, DT * B], F32R, tag="xf", name="xf")
                nc.vector.tensor_copy(xf[:], xf_raw[:])

                fs_ps = psm.tile([1, B], F32, tag="acc", space="PSUM", name="fs_ps")
                fq_ps = psm.tile([1, B], F32, tag="acc", space="PSUM", name="fq_ps")
                xfsq = spool.tile([P, DT * B], F32R, tag="xfsq", name="xfsq")
                nc.vector.tensor_mul(xfsq[:], xf[:], xf[:])
                for k in range(DT):
                    nc.tensor.matmul(fs_ps[:], ones_col[:], xf[:, k * B : (k + 1) * B],
                                     start=(k == 0), stop=(k == DT - 1))
                    nc.tensor.matmul(fq_ps[:], ones_col[:], xfsq[:, k * B : (k + 1) * B],
                                     start=(k == 0), stop=(k == DT - 1))
                fmean = spool.tile([1, B], F32, tag="lnstat", name="fmean")
                nc.vector.tensor_scalar_mul(fmean[:], fs_ps[:], 1.0 / D)
                fm2 = spool.tile([1, B], F32, tag="lnstat", name="fm2")
                nc.vector.tensor_mul(fm2[:], fmean[:], fmean[:])
                fsqd = spool.tile([1, B], F32, tag="lnstat", name="fsqd")
                nc.vector.tensor_scalar_mul(fsqd[:], fq_ps[:], 1.0 / D)
                fvar = spool.tile([1, B], F32, tag="lnstat", name="fvar")
                nc.vector.tensor_sub(fvar[:], fsqd[:], fm2[:])
                fstd = spool.tile([1, B], F32, tag="lnstat", name="fstd")
                nc.scalar.activation(fstd[:], fvar[:], AF.Sqrt, bias=eps1[:])
                frstd = spool.tile([1, B], F32, tag="lnr", name="frstd")
                nc.vector.reciprocal(frstd[:], fstd[:])
                frstd_r = spool.tile([1, B], F32R, tag="lnr", name="frstd_r")
                nc.vector.tensor_copy(frstd_r[:], frstd[:])
                fmrs = spool.tile([1, B], F32R, tag="lnr", name="fmrs")
                nc.vector.tensor_mul(fmrs[:], fmean[:], frstd[:])
                fr_ps = psr.tile([P, B], F32, tag="rot", space="PSUM", name="fr_ps")
                nc.tensor.matmul(fr_ps[:], ones_row[:], frstd_r[:],
                                 start=True, stop=True)
                fm_ps = psr.tile([P, B], F32, tag="rot", space="PSUM", name="fm_ps")
                nc.tensor.matmul(fm_ps[:], ones_row[:], fmrs[:],
                                 start=True, stop=True)
                xfn = spool.tile([P, DT * B], F16, tag="xfn", name="xfn")
                for k in range(DT):
                    t1 = spool.tile([P, B], F32, tag="lnstat", name="ft1")
                    nc.vector.tensor_mul(t1[:], xf[:, k * B : (k + 1) * B], fr_ps[:])
                    nc.vector.tensor_sub(xfn[:, k * B : (k + 1) * B], t1[:], fm_ps[:])

                for ci in range(VCH):
                    lg_ps = psr.tile([B, VCW], F32, tag="rot", space="PSUM", name="lg_ps")
                    for k in range(DT):
                        u_sb = ut_tiles[ci * DT + k][2]
                        nc.tensor.matmul(lg_ps[:], xfn[:, k * B : (k + 1) * B], u_sb[:],
                                         start=(k == 0), stop=(k == DT - 1))
                    och = fpool.tile([B, VCW], F32, tag="f2w", name="och")
                    nc.vector.tensor_copy(och[:], lg_ps[:])
                    nc.sync.dma_start(out[:, ci * VCW : (ci + 1) * VCW], och[:])

    nc.compile()
    return nc


# ---------------------------------------------------------------- host side
def _positional_encoding(s, d):
    idx = np.arange(d)
    exponent = ((2 * (idx // 2)).astype(np.float32) / float(d)).astype(np.float32)
    pos = np.arange(s, dtype=np.float32)[:, None]
    angle = pos / np.power(np.float32(10000.0), exponent[None, :], dtype=np.float32)
    return np.where((idx % 2 == 0)[None, :], np.sin(angle), np.cos(angle)).astype(np.float32)


def _build_masks():
    """trimask[r, c] = 1 if key r <= query c (within-block causal)."""
    r = np.arange(P)
    return (r[:, None] <= r[None, :]).astype(np.float16)


def prepare_inputs(tokens, predict_idx, embedding, ln1_g, ln1_b, wq, wk, wv, wo,
                   ln2_g, ln2_b, fc1, fc2, lnf_g, lnf_b, unembed, n_layers=NL):
    f = lambda a: np.ascontiguousarray(np.asarray(a), dtype=np.float32)
    tokens = np.asarray(tokens)
    emb = f(embedding)
    pos = _positional_encoding(S, D)

    ln1_g, ln1_b = f(ln1_g)[:n_layers], f(ln1_b)[:n_layers]
    ln2_g, ln2_b = f(ln2_g)[:n_layers], f(ln2_b)[:n_layers]
    lnf_gf, lnf_bf = f(lnf_g), f(lnf_b)

    def wlayout(a):  # [L, out, in] -> [L, P, DT*D] with [l, p, k*D + dout]
        aT = a.transpose(0, 2, 1)
        return np.ascontiguousarray(
            aT.reshape(n_layers, DT, P, D).transpose(0, 2, 1, 3)
            .reshape(n_layers, P, DT * D)).astype(np.float16)

    wq2 = f(wq)[:n_layers].reshape(-1, NH * DH, D)
    wk2 = f(wk)[:n_layers].reshape(-1, NH * DH, D)
    wv2 = f(wv)[:n_layers].reshape(-1, NH * DH, D)
    # fold LN1 gain into the QKV weight columns; betas become bias vectors
    wqT = wlayout(wq2 * ln1_g[:, None, :])
    wkT = wlayout(wk2 * ln1_g[:, None, :])
    wvT = wlayout(wv2 * ln1_g[:, None, :])
    woT = wlayout(f(wo)[:n_layers])
    cbq = np.einsum("lod,ld->lo", wq2, ln1_b)   # [L, 768]
    cbk = np.einsum("lod,ld->lo", wk2, ln1_b)
    cbv = np.einsum("lod,ld->lo", wv2, ln1_b)
    qkbias = np.stack([
        np.concatenate([cbq[l].reshape(DT, P).T, cbk[l].reshape(DT, P).T], axis=1)
        for l in range(n_layers)]).astype(np.float32)  # [L, P, 2*DT]
    vbias = cbv.reshape(n_layers, 1, NH * DH).astype(np.float16)

    fc1f = f(fc1)[:n_layers]
    fc2f = f(fc2)[:n_layers]
    fc1g = fc1f * ln2_g[:, None, :]
    fc1T = np.ascontiguousarray(
        fc1g.transpose(0, 2, 1)
        .reshape(n_layers, DT, P, FT, P).transpose(0, 3, 2, 1, 4)
        .reshape(n_layers, FT, P, DT * P)).astype(np.float16)
    fc2T = np.ascontiguousarray(
        fc2f.transpose(0, 2, 1)
        .reshape(n_layers, FT, P, D)).astype(np.float16)
    cbf = np.einsum("lod,ld->lo", fc1f, ln2_b)  # [L, 3072]
    fbias = np.ascontiguousarray(
        cbf.reshape(n_layers, FT, P).transpose(0, 2, 1)).astype(np.float32)

    uf = f(unembed) * lnf_gf[None, :]
    uTf = np.ascontiguousarray(uf.T.reshape(DT, P, V)).astype(np.float16)
    ubias = f(unembed) @ lnf_bf  # [V], added host-side

    bc2 = np.zeros((2, P), np.float32)
    bc2[0, :DH] = 1.0
    bc2[1, DH:] = 1.0

    masks = _build_masks()

    pidx = int(predict_idx)
    in_maps = []
    for c in range(NC_):
        b, h = c // 2, c % 2
        toks = np.asarray(tokens[b, h * T : (h + 1) * T]).astype(np.int64)
        x0 = emb.T[toks] + pos[h * T : (h + 1) * T]
        x0T = np.ascontiguousarray(x0.T.reshape(DT, P, T)).astype(np.float32)
        sel4 = np.zeros((P, B), np.float32)
        if pidx // T == h:
            sel4[:, b] = 1.0
        m = {
            "x0T": x0T, "wqT": wqT, "wkT": wkT, "wvT": wvT, "woT": woT,
            "fc1T": fc1T, "fc2T": fc2T,
            "qkb": qkbias, "vb": vbias, "fb": fbias,
            "uT": uTf[:, :, c * VS : (c + 1) * VS].copy(),
            "masks": masks,
            "remw": np.full((P, 1), 1.0 if h == 1 else 0.0, np.float32),
            "sel4": sel4,
            "pairsel": np.array([[1 - h]], np.int32),
            "bc2d": bc2,
        }
        in_maps.append(m)
    return in_maps, ubias


_CACHED = {}


def kernel(**inputs):
    from concourse.bass_utils import run_bass_kernel_spmd
    pidx = int(np.asarray(inputs["predict_idx"]))
    key = ("nc", pidx % T)
    if key not in _CACHED:
        _CACHED[key] = build_nc(pcol=pidx % T)
    nc = _CACHED[key]
    in_maps, ubias = prepare_inputs(**inputs)
    res = run_bass_kernel_spmd(nc, in_maps, core_ids=list(range(NC_)), trace=False)
    full = np.concatenate([res.results[c]["out"] for c in range(NC_)], axis=1)
    return full + ubias[None, :]
